# revision 46
# baseline (speedup 1.0000x reference)
"""BrainEncoder Trainium2 kernel.

Strategy
--------
Batch B=2048 is sorted by subject and split into 8 contiguous chunks of 256
samples (data-parallel, one chunk per NeuronCore).  Because the batch is
sorted, each chunk has one majority subject plus (for boundary chunks) a
minority run that host-side reordering confines to the tail <=128 samples.
The head is computed as W_base applied to all samples plus a masked
correction matmul with (W_min - W_base) over the tail window only.

On-device layout: positions are flattened as n = b*26 + j with j in [0,25)
holding t=0..24 and j=25 a zero pad shared between consecutive samples (the
right-pad of sample b doubles as the left-pad of sample b+1), plus one
leading/trailing pad column, so a SAME conv1d over T becomes 3 shifted
matmuls accumulated in PSUM.  Conv outputs land as [pos(128-part), channel]
tiles; gelu runs on ScalarE; LN mean/var come from DVE bn_stats, LN apply is
one tensor_scalar op.  LN gamma/beta are folded into the next conv's weights
on the host (with edge-row bias corrections realized as a K=3 indicator
matmul; this problem instance has all-zero biases so those matmuls are
skipped).  The normalized tile is transposed back to [channel, pos] on the
TensorEngine for the next layer.  The T-mean is a windowed selection matmul
into PSUM, scaled by 1/T during the psum->sbuf copy.

All matmuls run in bf16 (fp32 PSUM accumulation).  Output is DMA'd as bf16
and upcast on the host.
"""

import numpy as np
import ml_dtypes

import concourse.bass as bass
import concourse.bacc as bacc
import concourse.tile as tile
from concourse import mybir, masks
from concourse.bass_utils import run_bass_kernel_spmd

BF16 = ml_dtypes.bfloat16
E4M3 = ml_dtypes.float8_e4m3
f32 = mybir.dt.float32
bf16 = mybir.dt.bfloat16
f8e4 = mybir.dt.float8e4

# Problem constants (hardcoded per spec).
HID = 256
T = 25
VPF = 768
APF = 128
FD = 896           # frame dim = vid + aud
NSUBJ = 4
FMRI = 8192
B = 2048
NCORES = 8
BC = B // NCORES   # samples per core = 256
L = 26             # padded slots per sample (25 data + 1 shared pad)
NPOS = BC * L      # 6656 flat positions per core
WCOL = NPOS + 2    # + leading/trailing pad col
NT = NPOS // 128   # 52 position tiles
EPS = 1e-5
NCH1 = FD // 128   # 7 input-channel chunks for conv1
NCH = HID // 128   # 2 channel chunks for conv2/3
FBLK = 4           # head weight f-chunks of 2048
SELW = 8           # column window of the T-mean selection matmul
RSQRT_MAGIC = 0x5F3759DF


def _emit_program(bias_on=(True, True, True), s_tot=0) -> bass.Bass:
    nc = bacc.Bacc(None, target_bir_lowering=False, debug=False)
    AF = mybir.ActivationFunctionType
    OP = mybir.AluOpType
    DRM = mybir.MatmulPerfMode.DoubleRow

    x0_d = nc.declare_dram_parameter("x0", [FD, WCOL], bf16, isOutput=False)
    w1_d = nc.declare_dram_parameter("w1", [FD, 3, HID], bf16, isOutput=False)
    w2_d = nc.declare_dram_parameter("w2", [HID, 3, HID], bf16, isOutput=False)
    w3_d = nc.declare_dram_parameter("w3", [HID, 3, HID], bf16, isOutput=False)
    br1_d = nc.declare_dram_parameter("brow1", [3, HID], bf16, isOutput=False)
    br2_d = nc.declare_dram_parameter("brow2", [3, HID], bf16, isOutput=False)
    br3_d = nc.declare_dram_parameter("brow3", [3, HID], bf16, isOutput=False)
    ind_d = nc.declare_dram_parameter("ind", [3, WCOL], bf16, isOutput=False)
    hw_d = nc.declare_dram_parameter("hw", [2, HID, FMRI], bf16, isOutput=False)
    mk_d = nc.declare_dram_parameter("maskw", [128], bf16, isOutput=False)
    sel_d = nc.declare_dram_parameter("selw", [128, NT, SELW], bf16,
                                      isOutput=False)
    out_d = nc.declare_dram_parameter("out", [BC, FMRI], bf16, isOutput=True)

    with tile.TileContext(nc) as tc:
        from contextlib import ExitStack

        with ExitStack() as ctx:
            const = ctx.enter_context(tc.tile_pool(name="const", bufs=1))
            # conv1 fp8 input blocks (one tile per block, hi/lo interleaved)
            xbp = ctx.enter_context(tc.tile_pool(name="xbp", bufs=2))
            # head weight tiles (all 16 resident: streamed during conv2/3)
            xblk = ctx.enter_context(tc.tile_pool(name="xblk", bufs=16))
            zap = ctx.enter_context(tc.tile_pool(name="zap", bufs=4))
            sqp = ctx.enter_context(tc.tile_pool(name="sqp", bufs=3))
            stp = ctx.enter_context(tc.tile_pool(name="stp", bufs=10))
            hop = ctx.enter_context(tc.tile_pool(name="hop", bufs=2))
            xmp = ctx.enter_context(tc.tile_pool(name="xmp", bufs=1))
            # conv psum tiles and head output tiles share these 4 slots
            # (their phases only overlap at the conv3/head boundary).
            cps = ctx.enter_context(tc.tile_pool(name="cps", bufs=4, space="PSUM"))
            # transpose tiles (L1/L2)
            aux = ctx.enter_context(tc.tile_pool(name="aux", bufs=3, space="PSUM"))
            # T-mean accumulators live across the whole head phase -- one
            # bank of their own, out of the recycled slots.
            xsp = ctx.enter_context(tc.tile_pool(name="xsp", bufs=1, space="PSUM"))

            # ---- first conv1 input block + conv1 weights DMA first, so the
            # PE starts as early as possible; everything else queues after.
            block_list = [(0, 5), (5, 12), (12, 19), (19, 26),
                          (26, 33), (33, 40), (40, 46), (46, 52)]
            block_tiles = {}
            BW = 7 * 128 + 2

            def dma_block(k0, k1):
                w = 128 * (k1 - k0) + 2
                t = xbp.tile([128, NCH1, BW], bf16, name=f"x0b{k0}",
                             tag="xb8")
                for c in range(NCH1):
                    nc.sync.dma_start(t[:, c, :w],
                                      x0_d[128 * c:128 * (c + 1),
                                           128 * k0:128 * k1 + 2])
                block_tiles[k0] = t

            # interleave weight-chunk and first-block-chunk DMAs so chunk c's
            # first matmul can issue as soon as its own pair has landed
            k0, k1 = block_list[0]
            w = 128 * (k1 - k0) + 2
            w1_t = const.tile([128, NCH1, 3, HID], bf16, name="w1_t")
            bt0 = xbp.tile([128, NCH1, BW], bf16, name=f"x0b{k0}",
                           tag="xb8")
            for c in range(NCH1):
                nc.sync.dma_start(w1_t[:, c, :, :],
                                  w1_d[128 * c:128 * (c + 1), :, :])
                nc.sync.dma_start(bt0[:, c, :w],
                                  x0_d[128 * c:128 * (c + 1),
                                       128 * k0:128 * k1 + 2])
            block_tiles[k0] = bt0
            any_bias = any(bias_on)
            ind_t = None
            if any_bias:
                ind_t = const.tile([3, WCOL], bf16, name="ind_t")
                nc.sync.dma_start(ind_t[:], ind_d[:])
            br_t = []
            for li, brd in enumerate((br1_d, br2_d, br3_d)):
                t = const.tile([3, HID], bf16, name=f"br_{li}", tag=f"br_{li}")
                if bias_on[li]:
                    nc.sync.dma_start(t[:], brd[:])
                br_t.append(t)
            dma_block(*block_list[1])

            ident = const.tile([128, 128], bf16, name="ident")
            masks.make_identity(nc, ident[:])
            eps_t = const.tile([128, 1], f32, name="eps_t")
            nc.vector.memset(eps_t[:], EPS)
            zwarm = const.tile([128, 128], bf16, name="zwarm")
            nc.vector.memset(zwarm[:], 0.0)

            # PE clock warm-up: ~3us of dummy matmuls (no DMA deps) during
            # the initial input-DMA window, so the HAM ramp reaches 2.4GHz
            # before the first real conv tile instead of ~3us into it.
            warm_ps = cps.tile([128, 128], f32, name="warm_ps", tag="cps")
            NWARM = 24
            for i in range(NWARM):
                nc.tensor.matmul(warm_ps[:], lhsT=ident[:], rhs=zwarm[:],
                                 start=(i == 0), stop=(i == NWARM - 1),
                                 skip_group_check=True)
            warm_anchor = const.tile([128, 1], f32, name="warm_anchor")
            nc.vector.tensor_copy(out=warm_anchor[:], in_=warm_ps[:, 0:1])
            w2_t = []
            w3_t = []
            for c in range(NCH):
                t = const.tile([128, 3, HID], bf16, name=f"w2_{c}", tag=f"w2_{c}")
                nc.sync.dma_start(t[:], w2_d[128 * c:128 * (c + 1), :, :])
                w2_t.append(t)
                t = const.tile([128, 3, HID], bf16, name=f"w3_{c}", tag=f"w3_{c}")
                nc.sync.dma_start(t[:], w3_d[128 * c:128 * (c + 1), :, :])
                w3_t.append(t)

            # head correction mask, broadcast to 128 partitions
            mw_t = const.tile([128, 128], bf16, name="mw_t")
            nc.sync.dma_start(
                mw_t[:],
                bass.AP(tensor=mk_d.ap().tensor, offset=0,
                        ap=[[0, 128]] + list(mk_d.ap().ap)),
            )
            sel_t = const.tile([128, NT, SELW], bf16, name="sel_t")
            nc.sync.dma_start(sel_t[:], sel_d[:])

            # persistent activations
            yA = const.tile([128, NT, HID], bf16, name="yA")
            zb = []
            for i in range(2):
                pair = []
                for h in range(NCH):
                    t = const.tile([128, WCOL], bf16, name=f"zb{i}_{h}",
                                   tag=f"zb{i}_{h}")
                    # leading/trailing pad cols are never written again
                    nc.vector.memset(t[:, 0:1], 0.0)
                    nc.vector.memset(t[:, WCOL - 1:WCOL], 0.0)
                    pair.append(t)
                zb.append(pair)

            # ---- LN rstd via DVE-only Newton rsqrt (no ACT table switch) ----
            def rsqrt_half(li, half, MV, h0, W):
                u = stp.tile([128, W], f32, name=f"u{li}_{half}", tag="u")
                nc.vector.tensor_scalar(out=u[:], in0=MV[:, h0:h0 + W, 1],
                                        scalar1=EPS, scalar2=None, op0=OP.add)
                yi = stp.tile([128, W], mybir.dt.int32,
                              name=f"yi{li}_{half}", tag="yi")
                nc.vector.tensor_scalar(out=yi[:],
                                        in0=u[:].bitcast(mybir.dt.int32),
                                        scalar1=1, scalar2=None,
                                        op0=OP.arith_shift_right)
                nc.vector.tensor_scalar(out=yi[:], in0=yi[:],
                                        scalar1=-1, scalar2=RSQRT_MAGIC,
                                        op0=OP.mult, op1=OP.add)
                y0 = yi[:].bitcast(f32)
                a = stp.tile([128, W], f32, name=f"a{li}_{half}", tag="a")
                y1 = stp.tile([128, W], f32, name=f"y1{li}_{half}", tag="y1")
                rs = stp.tile([128, W], f32, name=f"rsq{li}_{half}", tag="rsq")
                for it, (src, dst) in enumerate(((y0, y1[:]), (y1[:], rs[:]))):
                    nc.vector.tensor_mul(out=a[:], in0=u[:], in1=src)
                    nc.vector.tensor_mul(out=a[:], in0=a[:], in1=src)
                    nc.vector.tensor_scalar(out=a[:], in0=a[:],
                                            scalar1=-0.5, scalar2=1.5,
                                            op0=OP.mult, op1=OP.add)
                    nc.vector.tensor_mul(out=dst, in0=src, in1=a[:])
                return rs

            # ---- per-half epilogue: normalize; then either transpose+copy to
            # zbuf (layers 1,2) or windowed sel-matmul T-mean accum (layer 3).
            def tail_half(li, half, MV, h0, h1, zbuf=None, xsum_ps=None):
                rs = rsqrt_half(li, half, MV, h0, h1 - h0)
                for k in range(h0, h1):
                    za = zap.tile([128, HID], bf16, name=f"za{li}_{k}", tag="za")
                    nc.vector.tensor_scalar(
                        out=za[:], in0=yA[:, k, :],
                        scalar1=MV[:, k, 0:1],
                        scalar2=rs[:, k - h0:k - h0 + 1],
                        op0=OP.subtract, op1=OP.mult)
                    if zbuf is not None:
                        for h in range(NCH):
                            tp = aux.tile([128, 128], bf16,
                                          name=f"tp{li}_{k}_{h}", tag="aux")
                            nc.tensor.transpose(
                                tp[:], za[:, 128 * h:128 * (h + 1)], ident[:])
                            if h == 0:
                                nc.vector.tensor_copy(
                                    out=zbuf[h][:, 1 + 128 * k:1 + 128 * (k + 1)],
                                    in_=tp[:])
                            else:
                                nc.scalar.copy(
                                    out=zbuf[h][:, 1 + 128 * k:1 + 128 * (k + 1)],
                                    in_=tp[:])
                    else:
                        c0 = min(128 * k // L, BC - SELW)
                        for h in range(NCH):
                            nc.tensor.matmul(
                                xsum_t[:, h, c0:c0 + SELW],
                                lhsT=za[:, 128 * h:128 * (h + 1)],
                                rhs=sel_t[:, k, :],
                                start=False, stop=(k == NT - 1),
                                skip_group_check=True)

            def pad_memsets(zbuf, half):
                # re-zero the j=25 pad slots that the copies overwrote
                # (half h covers samples [128h, 128h+128) = cols 3328h..)
                for h in range(NCH):
                    view = zbuf[h][:, 1 + 3328 * half:1 + 3328 * (half + 1)
                                   ].rearrange("p (b l) -> p b l", l=L)[:, :, T:L]
                    nc.vector.memset(view, 0.0)

            def conv_epilogue(li, k, ps, MV, scale=1.0):
                nc.scalar.activation(out=yA[:, k, :], in_=ps[:], func=AF.Gelu,
                                     scale=scale)
                st6 = sqp.tile([128, 6], f32, name=f"st{li}_{k}", tag="st6")
                nc.vector.bn_stats(out=st6[:], in_=yA[:, k, :])
                nc.vector.bn_aggr(out=MV[:, k, :], in_=st6[:])

            def conv_tile(li, k, ps, lhsT_fn, nchunk, wt, brow, MV):
                has_bias = bias_on[li - 1]
                for c in range(nchunk):
                    for d in range(3):
                        nc.tensor.matmul(ps[:], lhsT=lhsT_fn(c, d),
                                         rhs=wt[c][:, d, :],
                                         start=(c == 0 and d == 0),
                                         stop=(not has_bias and c == nchunk - 1
                                               and d == 2))
                if has_bias:
                    nc.tensor.matmul(
                        ps[:], lhsT=ind_t[:, 1 + 128 * k:1 + 128 * (k + 1)],
                        rhs=brow[:], start=False, stop=True)
                conv_epilogue(li, k, ps, MV)

            def conv1_tile(k, ps, xb, off):
                has_bias = bias_on[0]
                for c in range(NCH1):
                    for d in range(3):
                        nc.tensor.matmul(ps[:],
                                         lhsT=xb[:, c, off + d:off + d + 128],
                                         rhs=w1_t[:, c, d, :],
                                         start=(c == 0 and d == 0),
                                         stop=(not has_bias and c == NCH1 - 1
                                               and d == 2))
                if has_bias:
                    nc.tensor.matmul(
                        ps[:], lhsT=ind_t[:, 1 + 128 * k:1 + 128 * (k + 1)],
                        rhs=br_t[0][:], start=False, stop=True)
                conv_epilogue(1, k, ps, SQ1)

            HALF = NT // 2  # 26
            SQ1 = const.tile([128, NT, 2], f32, name="MV1")
            SQ2 = const.tile([128, NT, 2], f32, name="MV2")
            SQ3 = const.tile([128, NT, 2], f32, name="MV3")

            def conv1_range(kk0, kk1):
                for (k0, k1) in block_list:
                    if k1 <= kk0 or k0 >= kk1:
                        continue
                    if k0 not in block_tiles:
                        dma_block(k0, k1)
                    xb = block_tiles[k0]
                    for k in range(max(k0, kk0), min(k1, kk1)):
                        ps = cps.tile([128, HID], f32, name=f"ps1_{k}",
                                      tag="cps")
                        conv1_tile(k, ps, xb, 128 * (k - k0))

            def conv_range(li, zin, wt, brow, SQ, kk0, kk1):
                for k in range(kk0, kk1):
                    ps = cps.tile([128, HID], f32, name=f"ps{li}_{k}",
                                  tag="cps")

                    def lhsT_fn(c, d, _zin=zin, _k=k):
                        return _zin[c][:, 128 * _k + d:128 * _k + d + 128]

                    conv_tile(li, k, ps, lhsT_fn, NCH, wt, brow, SQ)

            # layer-3 T-mean accumulators, both halves in one PSUM bank
            xsum_t = xsp.tile([128, NCH, BC], f32, name="xsum", tag="xsum")
            xsum_ps = [xsum_t[:, h] for h in range(NCH)]

            # pooled features (T-mean via 1/T scale on the psum->sbuf copy),
            # plus the masked window-0 correction operand.
            xs = [xmp.tile([128, BC], bf16, name=f"xs{h}", tag=f"xs{h}")
                  for h in range(NCH)]
            xc = [xmp.tile([128, 128], bf16, name=f"xc{h}", tag=f"xc{h}")
                  for h in range(NCH)]

            def xs_copy(bk):
                sl = slice(128 * bk, 128 * (bk + 1))
                for h in range(NCH):
                    nc.scalar.activation(out=xs[h][:, sl],
                                         in_=xsum_t[:, h, sl],
                                         func=mybir.ActivationFunctionType.Copy,
                                         scale=1.0 / T)

            def corr_mul():
                for h in range(NCH):
                    nc.vector.tensor_mul(out=xc[h][:], in0=xs[h][:, 0:128],
                                         in1=mw_t[:])

            FQW = FMRI // FBLK

            def head_dma(fq):
                wt = {}
                for s in range(2):
                    for c in range(NCH):
                        t = xblk.tile([128, FQW], bf16,
                                      name=f"hw{fq}_{s}_{c}", tag="xb")
                        nc.sync.dma_start(
                            t[:], hw_d[s, 128 * c:128 * (c + 1),
                                       FQW * fq:FQW * (fq + 1)])
                        wt[(s, c)] = t
                return wt

            # ---- interleaved schedule: each layer's second-half epilogue is
            # emitted after the next layer's first conv tiles, so the PE has
            # dense work while the DVE computes LN stats.
            conv1_range(0, HALF)
            tail_half(1, 0, SQ1, 0, HALF, zbuf=zb[0])
            pad_memsets(zb[0], 0)
            conv1_range(HALF, NT)
            # all head weights start streaming now: the DMA engines are
            # otherwise idle during conv2/conv3.
            hw012 = [head_dma(0), head_dma(1), head_dma(2), head_dma(3)]
            conv_range(2, zb[0], w2_t, br_t[1], SQ2, 0, 13)
            tail_half(1, 1, SQ1, HALF, NT, zbuf=zb[0])
            pad_memsets(zb[0], 1)
            conv_range(2, zb[0], w2_t, br_t[1], SQ2, 13, HALF)
            tail_half(2, 0, SQ2, 0, HALF, zbuf=zb[1])
            pad_memsets(zb[1], 0)
            conv_range(2, zb[0], w2_t, br_t[1], SQ2, HALF, NT)
            conv_range(3, zb[1], w3_t, br_t[2], SQ3, 0, 13)
            tail_half(2, 1, SQ2, HALF, NT, zbuf=zb[1])
            pad_memsets(zb[1], 1)
            conv_range(3, zb[1], w3_t, br_t[2], SQ3, 13, HALF)
            nc.vector.memset(xsum_t[:], 0.0)
            tail_half(3, 0, SQ3, 0, HALF, xsum_ps=xsum_ps)
            conv_range(3, zb[1], w3_t, br_t[2], SQ3, HALF, NT)

            def head_group(fq, bk, wt):
                # minority samples sit in window 0, so the corrected (4-mm)
                # groups run early and the late bk=1 groups are 2 mm each.
                for fl in range(FQW // 512):
                    ft = (FQW // 512) * fq + fl
                    hp = cps.tile([128, 512], f32, name=f"hp{ft}_{bk}",
                                  tag="cps")
                    for c in range(NCH):
                        nc.tensor.matmul(
                            hp[:], lhsT=xs[c][:, 128 * bk:128 * (bk + 1)],
                            rhs=wt[(0, c)][:, 512 * fl:512 * (fl + 1)],
                            start=(c == 0),
                            stop=(c == NCH - 1 and bk == 1))
                    if bk == 0:
                        for c in range(NCH):
                            nc.tensor.matmul(
                                hp[:], lhsT=xc[c][:],
                                rhs=wt[(1, c)][:, 512 * fl:512 * (fl + 1)],
                                start=False, stop=(c == NCH - 1))
                    ho = hop.tile([128, 512], bf16, name=f"ho{ft}_{bk}",
                                  tag="ho")
                    nc.scalar.copy(out=ho[:], in_=hp[:])
                    nc.sync.dma_start(
                        out_d[128 * bk:128 * (bk + 1),
                              512 * ft:512 * (ft + 1)],
                        ho[:])

            xs_copy(0)
            corr_mul()
            head_group(0, 0, hw012[0])
            head_group(1, 0, hw012[1])
            tail_half(3, 1, SQ3, HALF, NT, xsum_ps=xsum_ps)
            xs_copy(1)
            head_group(0, 1, hw012[0])
            head_group(1, 1, hw012[1])
            head_group(2, 0, hw012[2])
            head_group(2, 1, hw012[2])
            head_group(3, 0, hw012[3])
            head_group(3, 1, hw012[3])
    return nc


_PROG_CACHE: dict[tuple, bass.Bass] = {}


def _get_program(bias_on=(True, True, True), s_tot=0) -> bass.Bass:
    key = (tuple(bias_on), s_tot)
    if key not in _PROG_CACHE:
        nc = _emit_program(tuple(bias_on), s_tot)
        nc.compile()
        _PROG_CACHE[key] = nc
    return _PROG_CACHE[key]


def _e4(a):
    return np.clip(a, -240.0, 240.0).astype(E4M3)


def _hilo(a):
    """Exact-ish hi/lo e4m3 split of a float array (a ~ O(1) scaled)."""
    hi = _e4(a)
    lo = _e4(a - hi.astype(np.float32))
    return hi, lo


def _pow2_scale(maxval):
    return int(np.clip(np.floor(np.log2(224.0 / max(maxval, 1e-12))), -20, 20))


def _host_prep(inputs):
    """Fold LN gamma/beta into downstream weights; build per-core arrays."""
    f8 = np.float64
    video = np.asarray(inputs["video"], np.float32)
    audio = np.asarray(inputs["audio"], np.float32)
    subj = np.asarray(inputs["subject_idx"]).astype(np.int64)
    cw = [np.asarray(inputs[f"conv{i}_w"], f8) for i in (1, 2, 3)]
    cb = [np.asarray(inputs[f"conv{i}_b"], f8) for i in (1, 2, 3)]
    g = [np.asarray(inputs[f"ln{i}_g"], f8) for i in (1, 2, 3)]
    bb = [np.asarray(inputs[f"ln{i}_b"], f8) for i in (1, 2, 3)]
    head_w = np.asarray(inputs["head_w"], f8)
    head_b = np.asarray(inputs["head_b"], f8)

    # conv1: no incoming fold
    w1r = cw[0].transpose(1, 2, 0)                     # [ci, d, co]
    br1 = np.stack([cb[0], np.zeros(HID), np.zeros(HID)])
    # conv2 <- ln1 fold, conv3 <- ln2 fold
    brows = [br1]
    wrs = [w1r]
    for i in (1, 2):
        Wf = cw[i] * g[i - 1][None, :, None]           # [co, ci, d]
        Sfull = np.einsum("ocd,c->o", cw[i], bb[i - 1])
        rL = -cw[i][:, :, 0] @ bb[i - 1]
        rR = -cw[i][:, :, 2] @ bb[i - 1]
        wrs.append(Wf.transpose(1, 2, 0))
        brows.append(np.stack([cb[i] + Sfull, rL, rR]))
    # head <- ln3 fold
    Wh = head_w * g[2][None, None, :]                  # [s, f, h]
    bh = np.einsum("sfh,h->sf", head_w, bb[2]) + head_b
    assert np.abs(bh).max() == 0.0, "nonzero head bias unsupported"

    # indicator rows over the padded column space
    ind = np.zeros((3, WCOL), np.float32)
    j = (np.arange(NPOS)) % L
    ind[0, 1:1 + NPOS] = 1.0
    ind[1, 1:1 + NPOS] = (j == 0)
    ind[2, 1:1 + NPOS] = (j == T - 1)

    # T-mean selection windows: sel[p, k, s - c0(k)] = 1 for valid t
    sel = np.zeros((128, NT, SELW), np.float32)
    for k in range(NT):
        c0 = min(128 * k // L, BC - SELW)
        gg = 128 * k + np.arange(128)
        s = gg // L
        t = gg % L
        valid = t < T
        sel[np.arange(128)[valid], k, (s - c0)[valid]] = 1.0

    perm = np.argsort(subj, kind="stable")
    frames = np.concatenate(
        [video.reshape(B, T, VPF), audio.reshape(B, T, APF)], axis=-1)
    s_tot = 0

    shared = {
        "w1": wrs[0].astype(BF16), "w2": wrs[1].astype(BF16),
        "w3": wrs[2].astype(BF16),
        "brow1": brows[0].astype(BF16), "brow2": brows[1].astype(BF16),
        "brow3": brows[2].astype(BF16),
        "ind": ind.astype(BF16),
        "selw": sel.astype(BF16),
    }
    bias_on = tuple(bool(np.abs(b).max() > 0) for b in brows)

    in_maps = []
    perm_out = np.empty(B, np.int64)
    for c in range(NCORES):
        idx = perm[c * BC:(c + 1) * BC]
        su = subj[idx]
        uniq, cnts = np.unique(su, return_counts=True)
        assert len(uniq) <= 2, "more than 2 subjects in a core chunk"
        maj = int(uniq[np.argmax(cnts)])
        # reorder: minority first, confined to the leading 128-sample window
        order = np.argsort(su == maj, kind="stable")
        idx = idx[order]
        qmin = int((su != maj).sum())
        assert qmin <= 128, "minority run exceeds one 128-sample window"
        perm_out[c * BC:(c + 1) * BC] = idx

        hw = np.zeros((2, HID, FMRI), np.float64)
        hw[0] = Wh[maj].T
        mask = np.zeros(128, np.float32)
        if qmin > 0:
            mn = int(uniq[np.argmin(cnts)])
            hw[1] = (Wh[mn] - Wh[maj]).T
            mask[:qmin] = 1.0

        fr = frames[idx]                                   # [BC, T, FD]
        x0 = np.zeros((FD, BC, L), np.float32)
        x0[:, :, 0:T] = fr.transpose(2, 0, 1)
        x0f = np.zeros((FD, WCOL), BF16)
        x0f[:, 1:1 + NPOS] = x0.reshape(FD, NPOS).astype(BF16)

        m = dict(shared)
        m.update({
            "x0": x0f, "hw": hw.astype(BF16), "maskw": mask.astype(BF16),
        })
        in_maps.append(m)
    return in_maps, perm_out, bias_on, s_tot


def kernel(**inputs) -> np.ndarray:
    in_maps, perm, bias_on, s_tot = _host_prep(inputs)
    nc = _get_program(bias_on, s_tot)
    res = run_bass_kernel_spmd(nc, in_maps, list(range(NCORES)))
    out = np.empty((B, FMRI), np.float32)
    for c in range(NCORES):
        out[perm[c * BC:(c + 1) * BC]] = \
            res.results[c]["out"].astype(np.float32)
    return out


# revision 54
# speedup vs baseline: 1.0271x; 1.0271x over previous
"""BrainEncoder Trainium2 kernel.

Strategy
--------
Batch B=2048 is sorted by subject and split into 8 contiguous chunks of 256
samples (data-parallel, one chunk per NeuronCore).  Because the batch is
sorted, each chunk has one majority subject plus (for boundary chunks) a
minority run that host-side reordering confines to the tail <=128 samples.
The head is computed as W_base applied to all samples plus a masked
correction matmul with (W_min - W_base) over the tail window only.

On-device layout: positions are flattened as n = b*26 + j with j in [0,25)
holding t=0..24 and j=25 a zero pad shared between consecutive samples (the
right-pad of sample b doubles as the left-pad of sample b+1), plus one
leading/trailing pad column, so a SAME conv1d over T becomes 3 shifted
matmuls accumulated in PSUM.  Conv outputs land as [pos(128-part), channel]
tiles; gelu runs on ScalarE; LN mean/var come from DVE bn_stats, LN apply is
one tensor_scalar op.  LN gamma/beta are folded into the next conv's weights
on the host (with edge-row bias corrections realized as a K=3 indicator
matmul; this problem instance has all-zero biases so those matmuls are
skipped).  The normalized tile is transposed back to [channel, pos] on the
TensorEngine for the next layer.  The T-mean is a windowed selection matmul
into PSUM, scaled by 1/T during the psum->sbuf copy.

All matmuls run in bf16 (fp32 PSUM accumulation).  Output is DMA'd as bf16
and upcast on the host.
"""

import numpy as np
import ml_dtypes

import concourse.bass as bass
import concourse.bacc as bacc
import concourse.tile as tile
from concourse import mybir, masks
from concourse.bass_utils import run_bass_kernel_spmd

BF16 = ml_dtypes.bfloat16
E4M3 = ml_dtypes.float8_e4m3
f32 = mybir.dt.float32
bf16 = mybir.dt.bfloat16
f8e4 = mybir.dt.float8e4

# Problem constants (hardcoded per spec).
HID = 256
T = 25
VPF = 768
APF = 128
FD = 896           # frame dim = vid + aud
NSUBJ = 4
FMRI = 8192
B = 2048
NCORES = 8
BC = B // NCORES   # samples per core = 256
L = 26             # padded slots per sample (25 data + 1 shared pad)
NPOS = BC * L      # 6656 flat positions per core
WCOL = NPOS + 2    # + leading/trailing pad col
NT = NPOS // 128   # 52 position tiles
EPS = 1e-5
NCH1 = FD // 128   # 7 input-channel chunks for conv1
NCH = HID // 128   # 2 channel chunks for conv2/3
FBLK = 4           # head weight f-chunks of 2048
SELW = 8           # column window of the T-mean selection matmul
RSQRT_MAGIC = 0x5F3759DF


def _emit_program(bias_on=(True, True, True), s_tot=0) -> bass.Bass:
    nc = bacc.Bacc(None, target_bir_lowering=False, debug=False)
    AF = mybir.ActivationFunctionType
    OP = mybir.AluOpType
    DRM = mybir.MatmulPerfMode.DoubleRow

    x0_d = nc.declare_dram_parameter("x0", [FD, WCOL], bf16, isOutput=False)
    w1_d = nc.declare_dram_parameter("w1", [FD, 3, HID], bf16, isOutput=False)
    w2_d = nc.declare_dram_parameter("w2", [HID, 3, HID], bf16, isOutput=False)
    w3_d = nc.declare_dram_parameter("w3", [HID, 3, HID], bf16, isOutput=False)
    br1_d = nc.declare_dram_parameter("brow1", [3, HID], bf16, isOutput=False)
    br2_d = nc.declare_dram_parameter("brow2", [3, HID], bf16, isOutput=False)
    br3_d = nc.declare_dram_parameter("brow3", [3, HID], bf16, isOutput=False)
    ind_d = nc.declare_dram_parameter("ind", [3, WCOL], bf16, isOutput=False)
    hw_d = nc.declare_dram_parameter("hw", [2, HID, FMRI], bf16, isOutput=False)
    mk_d = nc.declare_dram_parameter("maskw", [128], bf16, isOutput=False)
    sel_d = nc.declare_dram_parameter("selw", [128, NT, SELW], bf16,
                                      isOutput=False)
    out_d = nc.declare_dram_parameter("out", [BC, FMRI], bf16, isOutput=True)

    with tile.TileContext(nc) as tc:
        from contextlib import ExitStack

        with ExitStack() as ctx:
            const = ctx.enter_context(tc.tile_pool(name="const", bufs=1))
            # conv1 fp8 input blocks (one tile per block, hi/lo interleaved)
            xbp = ctx.enter_context(tc.tile_pool(name="xbp", bufs=2))
            # head weight tiles (all 16 resident: streamed during conv2/3)
            xblk = ctx.enter_context(tc.tile_pool(name="xblk", bufs=16))
            zap = ctx.enter_context(tc.tile_pool(name="zap", bufs=4))
            sqp = ctx.enter_context(tc.tile_pool(name="sqp", bufs=3))
            stp = ctx.enter_context(tc.tile_pool(name="stp", bufs=10))
            hop = ctx.enter_context(tc.tile_pool(name="hop", bufs=3))
            xmp = ctx.enter_context(tc.tile_pool(name="xmp", bufs=1))
            # conv psum: two [128,256] accumulation groups packed per bank
            cps = ctx.enter_context(tc.tile_pool(name="cps", bufs=2, space="PSUM"))
            # transpose tiles (L1/L2) and head output tiles
            aux = ctx.enter_context(tc.tile_pool(name="aux", bufs=5, space="PSUM"))
            # T-mean accumulators live across the whole head phase -- one
            # bank of their own, out of the recycled slots.
            xsp = ctx.enter_context(tc.tile_pool(name="xsp", bufs=1, space="PSUM"))

            # ---- first conv1 input block + conv1 weights DMA first, so the
            # PE starts as early as possible; everything else queues after.
            block_list = [(0, 5), (5, 12), (12, 19), (19, 26),
                          (26, 33), (33, 40), (40, 46), (46, 52)]
            block_tiles = {}
            BW = 7 * 128 + 2

            def dma_block(k0, k1):
                w = 128 * (k1 - k0) + 2
                t = xbp.tile([128, NCH1, BW], bf16, name=f"x0b{k0}",
                             tag="xb8")
                for c in range(NCH1):
                    nc.sync.dma_start(t[:, c, :w],
                                      x0_d[128 * c:128 * (c + 1),
                                           128 * k0:128 * k1 + 2])
                block_tiles[k0] = t

            # interleave weight-chunk and first-block-chunk DMAs so chunk c's
            # first matmul can issue as soon as its own pair has landed
            k0, k1 = block_list[0]
            w = 128 * (k1 - k0) + 2
            w1_t = const.tile([128, NCH1, 3, HID], bf16, name="w1_t")
            bt0 = xbp.tile([128, NCH1, BW], bf16, name=f"x0b{k0}",
                           tag="xb8")
            for c in range(NCH1):
                nc.sync.dma_start(w1_t[:, c, :, :],
                                  w1_d[128 * c:128 * (c + 1), :, :])
                nc.sync.dma_start(bt0[:, c, :w],
                                  x0_d[128 * c:128 * (c + 1),
                                       128 * k0:128 * k1 + 2])
            block_tiles[k0] = bt0
            any_bias = any(bias_on)
            ind_t = None
            if any_bias:
                ind_t = const.tile([3, WCOL], bf16, name="ind_t")
                nc.sync.dma_start(ind_t[:], ind_d[:])
            br_t = []
            for li, brd in enumerate((br1_d, br2_d, br3_d)):
                t = const.tile([3, HID], bf16, name=f"br_{li}", tag=f"br_{li}")
                if bias_on[li]:
                    nc.sync.dma_start(t[:], brd[:])
                br_t.append(t)
            dma_block(*block_list[1])

            ident = const.tile([128, 128], bf16, name="ident")
            masks.make_identity(nc, ident[:])
            eps_t = const.tile([128, 1], f32, name="eps_t")
            nc.vector.memset(eps_t[:], EPS)
            zwarm = const.tile([128, 128], bf16, name="zwarm")
            nc.vector.memset(zwarm[:], 0.0)

            # PE clock warm-up: ~3us of dummy matmuls (no DMA deps) during
            # the initial input-DMA window, so the HAM ramp reaches 2.4GHz
            # before the first real conv tile instead of ~3us into it.
            warm_ps = cps.tile([128, 128], f32, name="warm_ps", tag="cps")
            NWARM = 24
            for i in range(NWARM):
                nc.tensor.matmul(warm_ps[:], lhsT=ident[:], rhs=zwarm[:],
                                 start=(i == 0), stop=(i == NWARM - 1),
                                 skip_group_check=True)
            warm_anchor = const.tile([128, 1], f32, name="warm_anchor")
            nc.vector.tensor_copy(out=warm_anchor[:], in_=warm_ps[:, 0:1])
            w2_t = []
            w3_t = []
            for c in range(NCH):
                t = const.tile([128, 3, HID], bf16, name=f"w2_{c}", tag=f"w2_{c}")
                nc.sync.dma_start(t[:], w2_d[128 * c:128 * (c + 1), :, :])
                w2_t.append(t)
                t = const.tile([128, 3, HID], bf16, name=f"w3_{c}", tag=f"w3_{c}")
                nc.sync.dma_start(t[:], w3_d[128 * c:128 * (c + 1), :, :])
                w3_t.append(t)

            # head correction mask, broadcast to 128 partitions
            mw_t = const.tile([128, 128], bf16, name="mw_t")
            nc.sync.dma_start(
                mw_t[:],
                bass.AP(tensor=mk_d.ap().tensor, offset=0,
                        ap=[[0, 128]] + list(mk_d.ap().ap)),
            )
            sel_t = const.tile([128, NT, SELW], bf16, name="sel_t")
            nc.sync.dma_start(sel_t[:], sel_d[:])

            # persistent activations
            yA = const.tile([128, NT, HID], bf16, name="yA")
            zb = []
            for i in range(2):
                pair = []
                for h in range(NCH):
                    t = const.tile([128, WCOL], bf16, name=f"zb{i}_{h}",
                                   tag=f"zb{i}_{h}")
                    # leading/trailing pad cols are never written again
                    nc.vector.memset(t[:, 0:1], 0.0)
                    nc.vector.memset(t[:, WCOL - 1:WCOL], 0.0)
                    pair.append(t)
                zb.append(pair)

            # ---- LN rstd via DVE-only Newton rsqrt (no ACT table switch) ----
            def rsqrt_half(li, half, MV, h0, W):
                u = stp.tile([128, W], f32, name=f"u{li}_{half}", tag="u")
                nc.vector.tensor_scalar(out=u[:], in0=MV[:, h0:h0 + W, 1],
                                        scalar1=EPS, scalar2=None, op0=OP.add)
                yi = stp.tile([128, W], mybir.dt.int32,
                              name=f"yi{li}_{half}", tag="yi")
                nc.vector.tensor_scalar(out=yi[:],
                                        in0=u[:].bitcast(mybir.dt.int32),
                                        scalar1=1, scalar2=None,
                                        op0=OP.arith_shift_right)
                nc.vector.tensor_scalar(out=yi[:], in0=yi[:],
                                        scalar1=-1, scalar2=RSQRT_MAGIC,
                                        op0=OP.mult, op1=OP.add)
                y0 = yi[:].bitcast(f32)
                a = stp.tile([128, W], f32, name=f"a{li}_{half}", tag="a")
                y1 = stp.tile([128, W], f32, name=f"y1{li}_{half}", tag="y1")
                rs = stp.tile([128, W], f32, name=f"rsq{li}_{half}", tag="rsq")
                for it, (src, dst) in enumerate(((y0, y1[:]), (y1[:], rs[:]))):
                    nc.vector.tensor_mul(out=a[:], in0=u[:], in1=src)
                    nc.vector.tensor_mul(out=a[:], in0=a[:], in1=src)
                    nc.vector.tensor_scalar(out=a[:], in0=a[:],
                                            scalar1=-0.5, scalar2=1.5,
                                            op0=OP.mult, op1=OP.add)
                    nc.vector.tensor_mul(out=dst, in0=src, in1=a[:])
                return rs

            # ---- per-half epilogue: normalize; then either transpose+copy to
            # zbuf (layers 1,2) or windowed sel-matmul T-mean accum (layer 3).
            def tail_half(li, half, MV, h0, h1, zbuf=None, xsum_ps=None):
                rs = rsqrt_half(li, half, MV, h0, h1 - h0)
                for k in range(h0, h1):
                    za = zap.tile([128, HID], bf16, name=f"za{li}_{k}", tag="za")
                    nc.vector.tensor_scalar(
                        out=za[:], in0=yA[:, k, :],
                        scalar1=MV[:, k, 0:1],
                        scalar2=rs[:, k - h0:k - h0 + 1],
                        op0=OP.subtract, op1=OP.mult)
                    if zbuf is not None:
                        for h in range(NCH):
                            tp = aux.tile([128, 128], bf16,
                                          name=f"tp{li}_{k}_{h}", tag="aux")
                            nc.tensor.transpose(
                                tp[:], za[:, 128 * h:128 * (h + 1)], ident[:])
                            if h == 0:
                                nc.vector.tensor_copy(
                                    out=zbuf[h][:, 1 + 128 * k:1 + 128 * (k + 1)],
                                    in_=tp[:])
                            else:
                                nc.scalar.copy(
                                    out=zbuf[h][:, 1 + 128 * k:1 + 128 * (k + 1)],
                                    in_=tp[:])
                    else:
                        c0 = min(128 * k // L, BC - SELW)
                        for h in range(NCH):
                            nc.tensor.matmul(
                                xsum_t[:, h, c0:c0 + SELW],
                                lhsT=za[:, 128 * h:128 * (h + 1)],
                                rhs=sel_t[:, k, :],
                                start=False, stop=(k == NT - 1),
                                skip_group_check=True)

            def pad_memsets(zbuf, half):
                # re-zero the j=25 pad slots that the copies overwrote
                # (half h covers samples [128h, 128h+128) = cols 3328h..)
                for h in range(NCH):
                    view = zbuf[h][:, 1 + 3328 * half:1 + 3328 * (half + 1)
                                   ].rearrange("p (b l) -> p b l", l=L)[:, :, T:L]
                    nc.vector.memset(view, 0.0)

            def conv_epilogue(li, k, ps, MV, scale=1.0):
                nc.scalar.activation(out=yA[:, k, :], in_=ps, func=AF.Gelu,
                                     scale=scale)
                st6 = sqp.tile([128, 6], f32, name=f"st{li}_{k}", tag="st6")
                nc.vector.bn_stats(out=st6[:], in_=yA[:, k, :])
                nc.vector.bn_aggr(out=MV[:, k, :], in_=st6[:])

            def conv_tile(li, k, ps, lhsT_fn, nchunk, wt, brow, MV):
                has_bias = bias_on[li - 1]
                for c in range(nchunk):
                    for d in range(3):
                        nc.tensor.matmul(ps, lhsT=lhsT_fn(c, d),
                                         rhs=wt[c][:, d, :],
                                         start=(c == 0 and d == 0),
                                         stop=(not has_bias and c == nchunk - 1
                                               and d == 2))
                if has_bias:
                    nc.tensor.matmul(
                        ps, lhsT=ind_t[:, 1 + 128 * k:1 + 128 * (k + 1)],
                        rhs=brow[:], start=False, stop=True)
                conv_epilogue(li, k, ps, MV)

            def conv1_tile(k, ps, xb, off):
                has_bias = bias_on[0]
                for c in range(NCH1):
                    for d in range(3):
                        nc.tensor.matmul(ps,
                                         lhsT=xb[:, c, off + d:off + d + 128],
                                         rhs=w1_t[:, c, d, :],
                                         start=(c == 0 and d == 0),
                                         stop=(not has_bias and c == NCH1 - 1
                                               and d == 2))
                if has_bias:
                    nc.tensor.matmul(
                        ps, lhsT=ind_t[:, 1 + 128 * k:1 + 128 * (k + 1)],
                        rhs=br_t[0][:], start=False, stop=True)
                conv_epilogue(1, k, ps, SQ1)

            HALF = NT // 2  # 26
            SQ1 = const.tile([128, NT, 2], f32, name="MV1")
            SQ2 = const.tile([128, NT, 2], f32, name="MV2")
            SQ3 = const.tile([128, NT, 2], f32, name="MV3")

            def conv1_range(kk0, kk1):
                pt = [None]
                for (k0, k1) in block_list:
                    if k1 <= kk0 or k0 >= kk1:
                        continue
                    if k0 not in block_tiles:
                        dma_block(k0, k1)
                    xb = block_tiles[k0]
                    for k in range(max(k0, kk0), min(k1, kk1)):
                        j = k - kk0
                        if j % 2 == 0:
                            pt[0] = cps.tile([128, 2, HID], f32,
                                             name=f"ps1_{k}", tag="cps")
                        conv1_tile(k, pt[0][:, j % 2], xb, 128 * (k - k0))

            def conv_range(li, zin, wt, brow, SQ, kk0, kk1):
                pt = None
                for k in range(kk0, kk1):
                    j = k - kk0
                    if j % 2 == 0:
                        pt = cps.tile([128, 2, HID], f32, name=f"ps{li}_{k}",
                                      tag="cps")

                    def lhsT_fn(c, d, _zin=zin, _k=k):
                        return _zin[c][:, 128 * _k + d:128 * _k + d + 128]

                    conv_tile(li, k, pt[:, j % 2], lhsT_fn, NCH, wt, brow, SQ)

            # layer-3 T-mean accumulators, both halves in one PSUM bank
            xsum_t = xsp.tile([128, NCH, BC], f32, name="xsum", tag="xsum")
            xsum_ps = [xsum_t[:, h] for h in range(NCH)]

            # pooled features (T-mean via 1/T scale on the psum->sbuf copy),
            # plus the masked window-0 correction operand.
            xs = [xmp.tile([128, BC], bf16, name=f"xs{h}", tag=f"xs{h}")
                  for h in range(NCH)]
            xc = [xmp.tile([128, 128], bf16, name=f"xc{h}", tag=f"xc{h}")
                  for h in range(NCH)]

            def xs_copy(bk):
                sl = slice(128 * bk, 128 * (bk + 1))
                for h in range(NCH):
                    nc.scalar.activation(out=xs[h][:, sl],
                                         in_=xsum_t[:, h, sl],
                                         func=mybir.ActivationFunctionType.Copy,
                                         scale=1.0 / T)

            def corr_mul():
                for h in range(NCH):
                    nc.vector.tensor_mul(out=xc[h][:], in0=xs[h][:, 0:128],
                                         in1=mw_t[:])

            FQW = FMRI // FBLK

            def head_dma(fq):
                wt = {}
                for s in range(2):
                    for c in range(NCH):
                        t = xblk.tile([128, FQW], bf16,
                                      name=f"hw{fq}_{s}_{c}", tag="xb")
                        nc.sync.dma_start(
                            t[:], hw_d[s, 128 * c:128 * (c + 1),
                                       FQW * fq:FQW * (fq + 1)])
                        wt[(s, c)] = t
                return wt

            # ---- interleaved schedule: each layer's second-half epilogue is
            # emitted after the next layer's first conv tiles, so the PE has
            # dense work while the DVE computes LN stats.
            conv1_range(0, HALF)
            tail_half(1, 0, SQ1, 0, HALF, zbuf=zb[0])
            pad_memsets(zb[0], 0)
            conv1_range(HALF, NT)
            # all head weights start streaming now: the DMA engines are
            # otherwise idle during conv2/conv3.
            hw012 = [head_dma(0), head_dma(1), head_dma(2), head_dma(3)]
            conv_range(2, zb[0], w2_t, br_t[1], SQ2, 0, 13)
            tail_half(1, 1, SQ1, HALF, NT, zbuf=zb[0])
            pad_memsets(zb[0], 1)
            conv_range(2, zb[0], w2_t, br_t[1], SQ2, 13, HALF)
            tail_half(2, 0, SQ2, 0, HALF, zbuf=zb[1])
            pad_memsets(zb[1], 0)
            conv_range(2, zb[0], w2_t, br_t[1], SQ2, HALF, NT)
            conv_range(3, zb[1], w3_t, br_t[2], SQ3, 0, 13)
            tail_half(2, 1, SQ2, HALF, NT, zbuf=zb[1])
            pad_memsets(zb[1], 1)
            conv_range(3, zb[1], w3_t, br_t[2], SQ3, 13, HALF)
            nc.vector.memset(xsum_t[:], 0.0)
            tail_half(3, 0, SQ3, 0, HALF, xsum_ps=xsum_ps)
            conv_range(3, zb[1], w3_t, br_t[2], SQ3, HALF, NT)

            def head_group(fq, bk, wt):
                # minority samples sit in window 0, so the corrected (4-mm)
                # groups run early and the late bk=1 groups are 2 mm each.
                # fl-pairs share one [128,1024] output DMA; the psum->sbuf
                # copies alternate ACT/DVE so neither chains the phase.
                for fp in range(FQW // 1024):
                    ho = hop.tile([128, 2, 512], bf16,
                                  name=f"ho{fq}_{fp}_{bk}", tag="ho")
                    for half in range(2):
                        fl = 2 * fp + half
                        ft = (FQW // 512) * fq + fl
                        hp = aux.tile([128, 512], f32, name=f"hp{ft}_{bk}",
                                      tag="aux")
                        for c in range(NCH):
                            nc.tensor.matmul(
                                hp[:], lhsT=xs[c][:, 128 * bk:128 * (bk + 1)],
                                rhs=wt[(0, c)][:, 512 * fl:512 * (fl + 1)],
                                start=(c == 0),
                                stop=(c == NCH - 1 and bk == 1))
                        if bk == 0:
                            for c in range(NCH):
                                nc.tensor.matmul(
                                    hp[:], lhsT=xc[c][:],
                                    rhs=wt[(1, c)][:, 512 * fl:512 * (fl + 1)],
                                    start=False, stop=(c == NCH - 1))
                        if half == 0:
                            nc.scalar.copy(out=ho[:, 0], in_=hp[:])
                        else:
                            nc.vector.tensor_copy(out=ho[:, 1], in_=hp[:])
                    nc.sync.dma_start(
                        out_d[128 * bk:128 * (bk + 1),
                              1024 * ((FQW // 1024) * fq + fp):
                              1024 * ((FQW // 1024) * fq + fp + 1)],
                        ho[:].rearrange("p a b -> p (a b)"))

            xs_copy(0)
            corr_mul()
            for fq in range(FBLK):
                head_group(fq, 0, hw012[fq])
            tail_half(3, 1, SQ3, HALF, NT, xsum_ps=xsum_ps)
            xs_copy(1)
            for fq in range(FBLK):
                head_group(fq, 1, hw012[fq])
    return nc


_PROG_CACHE: dict[tuple, bass.Bass] = {}


def _get_program(bias_on=(True, True, True), s_tot=0) -> bass.Bass:
    key = (tuple(bias_on), s_tot)
    if key not in _PROG_CACHE:
        nc = _emit_program(tuple(bias_on), s_tot)
        nc.compile()
        _PROG_CACHE[key] = nc
    return _PROG_CACHE[key]


def _e4(a):
    return np.clip(a, -240.0, 240.0).astype(E4M3)


def _hilo(a):
    """Exact-ish hi/lo e4m3 split of a float array (a ~ O(1) scaled)."""
    hi = _e4(a)
    lo = _e4(a - hi.astype(np.float32))
    return hi, lo


def _pow2_scale(maxval):
    return int(np.clip(np.floor(np.log2(224.0 / max(maxval, 1e-12))), -20, 20))


def _host_prep(inputs):
    """Fold LN gamma/beta into downstream weights; build per-core arrays."""
    f8 = np.float64
    video = np.asarray(inputs["video"], np.float32)
    audio = np.asarray(inputs["audio"], np.float32)
    subj = np.asarray(inputs["subject_idx"]).astype(np.int64)
    cw = [np.asarray(inputs[f"conv{i}_w"], f8) for i in (1, 2, 3)]
    cb = [np.asarray(inputs[f"conv{i}_b"], f8) for i in (1, 2, 3)]
    g = [np.asarray(inputs[f"ln{i}_g"], f8) for i in (1, 2, 3)]
    bb = [np.asarray(inputs[f"ln{i}_b"], f8) for i in (1, 2, 3)]
    head_w = np.asarray(inputs["head_w"], f8)
    head_b = np.asarray(inputs["head_b"], f8)

    # conv1: no incoming fold
    w1r = cw[0].transpose(1, 2, 0)                     # [ci, d, co]
    br1 = np.stack([cb[0], np.zeros(HID), np.zeros(HID)])
    # conv2 <- ln1 fold, conv3 <- ln2 fold
    brows = [br1]
    wrs = [w1r]
    for i in (1, 2):
        Wf = cw[i] * g[i - 1][None, :, None]           # [co, ci, d]
        Sfull = np.einsum("ocd,c->o", cw[i], bb[i - 1])
        rL = -cw[i][:, :, 0] @ bb[i - 1]
        rR = -cw[i][:, :, 2] @ bb[i - 1]
        wrs.append(Wf.transpose(1, 2, 0))
        brows.append(np.stack([cb[i] + Sfull, rL, rR]))
    # head <- ln3 fold
    Wh = head_w * g[2][None, None, :]                  # [s, f, h]
    bh = np.einsum("sfh,h->sf", head_w, bb[2]) + head_b
    assert np.abs(bh).max() == 0.0, "nonzero head bias unsupported"

    # indicator rows over the padded column space
    ind = np.zeros((3, WCOL), np.float32)
    j = (np.arange(NPOS)) % L
    ind[0, 1:1 + NPOS] = 1.0
    ind[1, 1:1 + NPOS] = (j == 0)
    ind[2, 1:1 + NPOS] = (j == T - 1)

    # T-mean selection windows: sel[p, k, s - c0(k)] = 1 for valid t
    sel = np.zeros((128, NT, SELW), np.float32)
    for k in range(NT):
        c0 = min(128 * k // L, BC - SELW)
        gg = 128 * k + np.arange(128)
        s = gg // L
        t = gg % L
        valid = t < T
        sel[np.arange(128)[valid], k, (s - c0)[valid]] = 1.0

    perm = np.argsort(subj, kind="stable")
    frames = np.concatenate(
        [video.reshape(B, T, VPF), audio.reshape(B, T, APF)], axis=-1)
    s_tot = 0

    shared = {
        "w1": wrs[0].astype(BF16), "w2": wrs[1].astype(BF16),
        "w3": wrs[2].astype(BF16),
        "brow1": brows[0].astype(BF16), "brow2": brows[1].astype(BF16),
        "brow3": brows[2].astype(BF16),
        "ind": ind.astype(BF16),
        "selw": sel.astype(BF16),
    }
    bias_on = tuple(bool(np.abs(b).max() > 0) for b in brows)

    in_maps = []
    perm_out = np.empty(B, np.int64)
    for c in range(NCORES):
        idx = perm[c * BC:(c + 1) * BC]
        su = subj[idx]
        uniq, cnts = np.unique(su, return_counts=True)
        assert len(uniq) <= 2, "more than 2 subjects in a core chunk"
        maj = int(uniq[np.argmax(cnts)])
        # reorder: minority first, confined to the leading 128-sample window
        order = np.argsort(su == maj, kind="stable")
        idx = idx[order]
        qmin = int((su != maj).sum())
        assert qmin <= 128, "minority run exceeds one 128-sample window"
        perm_out[c * BC:(c + 1) * BC] = idx

        hw = np.zeros((2, HID, FMRI), np.float64)
        hw[0] = Wh[maj].T
        mask = np.zeros(128, np.float32)
        if qmin > 0:
            mn = int(uniq[np.argmin(cnts)])
            hw[1] = (Wh[mn] - Wh[maj]).T
            mask[:qmin] = 1.0

        fr = frames[idx]                                   # [BC, T, FD]
        x0 = np.zeros((FD, BC, L), np.float32)
        x0[:, :, 0:T] = fr.transpose(2, 0, 1)
        x0f = np.zeros((FD, WCOL), BF16)
        x0f[:, 1:1 + NPOS] = x0.reshape(FD, NPOS).astype(BF16)

        m = dict(shared)
        m.update({
            "x0": x0f, "hw": hw.astype(BF16), "maskw": mask.astype(BF16),
        })
        in_maps.append(m)
    return in_maps, perm_out, bias_on, s_tot


def kernel(**inputs) -> np.ndarray:
    in_maps, perm, bias_on, s_tot = _host_prep(inputs)
    nc = _get_program(bias_on, s_tot)
    res = run_bass_kernel_spmd(nc, in_maps, list(range(NCORES)))
    out = np.empty((B, FMRI), np.float32)
    for c in range(NCORES):
        out[perm[c * BC:(c + 1) * BC]] = \
            res.results[c]["out"].astype(np.float32)
    return out
